# revision 7
# baseline (speedup 1.0000x reference)
"""CMSBlockLinear block-ELL sparse linear forward on 8 trn2 NeuronCores.

Strategy: the block-sparse weight (R=128 x K=32 active 16x16 tiles, 25%
density) is densified on the host into W^T [2048 in, 2048 out].  The device
runs a dense matmul y^T = W^T.T @ x^T with fp32 PSUM accumulation.
Dense-ifying costs 4x the weight FLOPs on paper, but the PE streams N
columns per matmul regardless of M, so a dense 128-wide M uses the array 8x
better than the natural M=16 sparse formulation.

Sharding (8 cores): 4-way over tokens x 2-way over output features.

v2 numeric config (error budget vs the 2e-2 gate, host-sim calibrated to
5 digits against HW at the previous 2-chunk config):
- Contraction chunks 0-3 ride in fp8(e4m3) as TWO DoubleRow pair passes
  (2 k-tiles per instruction, double-pumped PE): 4 chunks of progress for
  2 chunks of PE cycles and half the DMA bytes.
- Everything else (chunks 4-15, and the output y) is fp16 instead of bf16:
  same PE rate and DMA bytes, 8x less quantization error, which buys the
  margin for the second fp8 pair.  w8 is scaled x4 and x8 by 1/4 (exact
  powers of two, product invariant) to pull w's small values out of e4m3's
  subnormal range.  Host-sim predicted rel_err 1.897e-2.

Device loop (trace-driven rework of the 44.4us v1):
- Input DMAs are need-ordered on the two HWDGE rings: sync carries
  [x8a(pair1), w8R, x8b(pair2), then x chunks 4..15], scalar carries
  [w8L, then w chunks 4..15].  w8 is split by output half so pass A's
  m=0..3 only waits on w8L+x8a; x8b/w8R land during pass A so pass B
  follows gaplessly (v1 lost 938ns waiting for its second fp8 piece).
- Warm-up dummy matmuls hold the PE (and ramp the DVFS clock) until the
  first data lands.  v1's 10-deep chain overshot the data by ~1.9us at
  the cold 1.2GHz clock; 6 slots end ~10.6us, right at the measured
  first-sem window.
- Steady state is per-chunk demand-paced fp16 DMAs (x on sync, w on
  scalar, buffers rotating 5/6-deep): front-loading everything at once
  keeps ~300GB/s in flight through the clock ramp and the DVFS governor
  then parks the PE at 2.0GHz instead of 2.4 for the whole stream.
- bias is applied on the host (zeros in this problem, exact in fp32
  either way).  Epilogue: last three chunks m-major so bank m closes
  ~0.65us before bank m+1 and the psum copies (even m on DVE, odd m on
  Scalar-ACT) + output DMAs hide under the stream tail; m7's final piece
  has its copy AND its DMA each split across both engines/rings.
- ~7.2us NEFF entry + ~3us exit (semaphore resets/barriers) are inside
  the measured window and fixed.
"""

import os

import numpy as np

BATCH, SEQ = 4, 512
IN_F = OUT_F = 2048
B = 16
R = 128  # output block rows
C = 128  # input block cols
KBLK = 32  # active tiles per row

TOK = BATCH * SEQ  # 2048 tokens
TOK_SHARDS = 4
OUT_SHARDS = 2
TOK_PER = TOK // TOK_SHARDS  # 512
OUT_PER = OUT_F // OUT_SHARDS  # 1024
K_CHUNKS = IN_F // 128  # 16
M_CHUNKS = OUT_PER // 128  # 8
N_FP8 = 4  # contraction chunks 0..3 in fp8 as two DoubleRow pairs
FP8_SCALE = 4.0  # w8 *= 4, x8 /= 4: rebalance e4m3 subnormal loss

# Warm slots bridge the first-DMA wait: the chain runs at the cold ~1.2GHz
# clock (~427ns per 512-wide matmul).  Measured body-relative: PE enters the
# body at ~body+0.65us and pass-A data is consumable at ~body+4.4us (queue
# start ~body+1.7, critical 256KB, ~1.9us completion-sem latency), so 8
# slots (~3.6us) bridge it.  PE idle mid-ramp delays the 2.4GHz grant by
# about the idle length, so the chain should end at (not before) arrival.
N_WARM = 8

LAST_EXEC_TIME_NS = None

_CACHE = {}


def _ensure_profile_hook():
    """Provide antenv.axon_hooks if the image lacks it, so trace=True works.

    Mirrors trn_agent_boot._ntff_profile_via_ctypes: drives NTFF capture via
    the libaxon_pjrt.so C ABI.  Also makes upload_artifacts fall back to the
    local dir when no artifact store is reachable.
    """
    import contextlib
    import ctypes
    import sys
    import types

    try:
        import antenv.axon_hooks  # noqa: F401

        return
    except ImportError:
        pass

    so_path = "/opt/axon/libaxon_pjrt.so"
    _hook = None
    if os.path.exists(so_path):
        try:
            lib = ctypes.CDLL(so_path)
            if hasattr(lib, "axon_start_nrt_profile"):
                lib.axon_start_nrt_profile.argtypes = [
                    ctypes.POINTER(ctypes.c_int64),
                    ctypes.c_size_t,
                ]
                lib.axon_start_nrt_profile.restype = ctypes.c_int64
                lib.axon_stop_nrt_profile.argtypes = [ctypes.c_char_p]
                lib.axon_stop_nrt_profile.restype = ctypes.c_int64

                @contextlib.contextmanager
                def _ntff_hook(output_dir, device_ids):
                    import jax

                    jax.devices()
                    if device_ids:
                        ids = (ctypes.c_int64 * len(device_ids))(*device_ids)
                        rc = lib.axon_start_nrt_profile(ids, len(device_ids))
                    else:
                        rc = lib.axon_start_nrt_profile(None, 0)
                    if rc != 0:
                        raise RuntimeError(f"axon_start_nrt_profile rc={rc}")
                    try:
                        yield
                    finally:
                        n = lib.axon_stop_nrt_profile(str(output_dir).encode())
                        print(f"profile: {n} file(s) -> {output_dir}", file=sys.stderr)

                _hook = _ntff_hook
        except OSError:
            pass

    mod = types.ModuleType("antenv.axon_hooks")
    mod.get_axon_ntff_profile_hook = lambda: _hook
    sys.modules["antenv.axon_hooks"] = mod

    import concourse.bass_utils as _bu

    _orig_upload = _bu.upload_artifacts

    def _safe_upload(tmpdir):
        try:
            return _orig_upload(tmpdir)
        except Exception:
            return tmpdir

    _bu.upload_artifacts = _safe_upload


def _build_nc():
    import concourse.mybir as mybir
    from concourse import bacc
    from concourse.tile import TileContext

    f16 = mybir.dt.float16
    fp8 = mybir.dt.float8e4

    nc = bacc.Bacc("TRN2", target_bir_lowering=False)
    xT = nc.dram_tensor("xT", [IN_F, TOK_PER], f16, kind="ExternalInput")
    w = nc.dram_tensor("w", [IN_F, OUT_PER], f16, kind="ExternalInput")
    # fp8 pieces.  Layout [p, t, :] with t = chunk index within the group;
    # lhsT/rhs agree so the DoubleRow (p, t) reduction maps correctly.
    # w8 is split by output-column half so pass A's first matmuls only wait
    # on w8L; x8 by pair so pass A only waits on x8a.
    x8a = nc.dram_tensor("x8a", [128, 2, TOK_PER], fp8, kind="ExternalInput")
    x8b = nc.dram_tensor("x8b", [128, 2, TOK_PER], fp8, kind="ExternalInput")
    Q = OUT_PER // 4  # 256 output cols = 2 m-chunks per w8 quarter piece
    w8La = nc.dram_tensor("w8La", [128, N_FP8, Q], fp8, kind="ExternalInput")
    w8Lb = nc.dram_tensor("w8Lb", [128, N_FP8, Q], fp8, kind="ExternalInput")
    w8R = nc.dram_tensor("w8R", [128, N_FP8, OUT_PER // 2], fp8, kind="ExternalInput")
    # y device layout: [partition, col-group, token] with col-groups
    # [m0,m2,m4,m6,m1,m3,m5,m7] - 1-2 KB contiguous per (partition, push).
    # Host un-permutes.
    y = nc.dram_tensor("y", [128, M_CHUNKS * TOK_PER], f16, kind="ExternalOutput")

    with TileContext(nc) as tc:
        with (
            tc.tile_pool(name="consts", bufs=1) as consts,
            tc.tile_pool(name="xp", bufs=5) as xp,
            tc.tile_pool(name="wp", bufs=6) as wp,
            tc.tile_pool(name="op", bufs=1) as op,
            tc.tile_pool(name="ps", bufs=1, space="PSUM") as ps,
        ):
            psums = [
                ps.tile([128, TOK_PER], mybir.dt.float32, tag=f"ps{m}", name=f"ps{m}")
                for m in range(M_CHUNKS)
            ]

            # Warm-up: dummy matmuls hold the PE busy (and ramp the DVFS
            # clock) until pass A's data lands.  Contents irrelevant (pass
            # A's start=True resets each bank), but Tile needs a writer to
            # allocate the tile - one cheap column memset suffices.
            warm = consts.tile([128, TOK_PER], f16)
            nc.vector.memset(warm[:, :1], 0)
            for i in range(N_WARM):
                nc.tensor.matmul(
                    psums[0][:],
                    warm[:, :128],
                    warm[:],
                    start=(i == 0),
                    stop=(i == N_WARM - 1),
                )

            # Input DMAs, need-ordered so pass A's first matmuls gate on the
            # smallest possible set (x8a on sync + w8La on scalar, 256KB):
            # sync carries [x8a, then x chunks], scalar carries [w8La, w8Lb,
            # w8R, x8b, then w chunks] in consumption order.  Every chunk in
            # its own resident buffer, rotation 5/6-deep paces the stream
            # for DVFS.
            x8at = xp.tile([128, 2, TOK_PER], fp8, name="x8at", tag="x8at")
            x8bt = xp.tile([128, 2, TOK_PER], fp8, name="x8bt", tag="x8bt")
            w8Lat = wp.tile([128, N_FP8, Q], fp8, name="w8Lat", tag="w8Lat")
            w8Lbt = wp.tile([128, N_FP8, Q], fp8, name="w8Lbt", tag="w8Lbt")
            w8Rt = wp.tile([128, N_FP8, OUT_PER // 2], fp8, name="w8Rt", tag="w8Rt")
            nc.sync.dma_start(x8at[:], x8a[:])
            nc.scalar.dma_start(w8Lat[:], w8La[:])
            nc.scalar.dma_start(w8Lbt[:], w8Lb[:])
            nc.scalar.dma_start(w8Rt[:], w8R[:])
            nc.scalar.dma_start(x8bt[:], x8b[:])
            xks, wks = [], []
            for k in range(K_CHUNKS):
                if k < N_FP8:
                    xks.append(None)
                    wks.append(None)
                    continue
                xk = xp.tile([128, TOK_PER], f16, name=f"xk{k}", tag="xk")
                wk = wp.tile([128, OUT_PER], f16, name=f"wk{k}", tag="wk")
                nc.sync.dma_start(xk[:], xT[k * 128 : (k + 1) * 128, :])
                nc.scalar.dma_start(wk[:], w[k * 128 : (k + 1) * 128, :])
                xks.append(xk)
                wks.append(wk)

            # fp8 stream opener: two DoubleRow pair passes (chunks 0-1 then
            # 2-3), full-token.  Pass A's start=True clears each bank.
            for pair, x8t in ((0, x8at), (1, x8bt)):
                t0 = 2 * pair
                for m in range(M_CHUNKS):
                    w8t = (w8Lat, w8Lbt, w8Rt, w8Rt)[m // 2]
                    mm = m % 2 if m < 4 else m % 4
                    nc.tensor.matmul(
                        psums[m][:],
                        w8t[:, t0 : t0 + 2, mm * 128 : (mm + 1) * 128],
                        x8t[:],
                        start=(pair == 0),
                        stop=False,
                        perf_mode=mybir.MatmulPerfMode.DoubleRow,
                    )
            # Steady state: k-outer, m-inner (fp16 chunks 4..12).
            for k in range(N_FP8, K_CHUNKS - 3):
                for m in range(M_CHUNKS):
                    nc.tensor.matmul(
                        psums[m][:],
                        wks[k][:, m * 128 : (m + 1) * 128],
                        xks[k][:],
                        start=False,
                        stop=False,
                    )

            outA = op.tile([128, M_CHUNKS // 2, TOK_PER], f16, name="outA")
            outB = op.tile([128, M_CHUNKS // 2, TOK_PER], f16, name="outB")

            # Epilogue: last three chunks m-major so bank m closes ~0.65us
            # before bank m+1; each bank's copy and each output DMA push is
            # emitted right behind its close and overlaps the stream tail.
            T = TOK_PER
            H = TOK_PER // 2
            for m in range(M_CHUNKS):
                for kk in range(K_CHUNKS - 3, K_CHUNKS):
                    nc.tensor.matmul(
                        psums[m][:],
                        wks[kk][:, m * 128 : (m + 1) * 128],
                        xks[kk][:],
                        start=False,
                        stop=(kk == K_CHUNKS - 1),
                    )
                j = m // 2
                if m == M_CHUNKS - 1:
                    # Split the last bank's copy across both engines to
                    # halve the post-stream copy latency.
                    nc.vector.tensor_scalar_add(outB[:, j, 0:H], psums[m][:, 0:H], 0.0)
                    nc.scalar.copy(outB[:, j, H:T], psums[m][:, H:T])
                elif m % 2 == 0:
                    nc.vector.tensor_scalar_add(outA[:, j, :], psums[m][:], 0.0)
                else:
                    nc.scalar.copy(outB[:, j, :], psums[m][:])

                if m == 2:
                    nc.sync.dma_start(y[:, 0 : 2 * T], outA[:, 0:2, :])  # m0,m2
                elif m == 3:
                    nc.scalar.dma_start(y[:, 4 * T : 6 * T], outB[:, 0:2, :])  # m1,m3
                elif m == 5:
                    nc.sync.dma_start(y[:, 6 * T : 7 * T], outB[:, 2:3, :])  # m5
                elif m == 6:
                    nc.sync.dma_start(y[:, 2 * T : 4 * T], outA[:, 2:4, :])  # m4,m6
                elif m == M_CHUNKS - 1:
                    # m7's 128 KB split across both rings, each half pushed
                    # right behind its own copy, so the final drain is two
                    # parallel 64 KB transfers instead of one serial piece.
                    nc.sync.dma_start(y[:, 7 * T : 7 * T + H], outB[:, 3, 0:H])
                    nc.scalar.dma_start(y[:, 7 * T + H : 8 * T], outB[:, 3, H:T])

    nc.finalize()
    return nc


def _densify_wT(values: np.ndarray, col_indices: np.ndarray) -> np.ndarray:
    """W^T [in=2048, out=2048] with W[r*16+i, c*16+j] = values[r,k,i,j]."""
    wT = np.zeros((C, B, R, B), dtype=np.float32)  # [c, j, r, i]
    vals_t = values.transpose(0, 1, 3, 2)  # [R, K, j, i]
    r_idx = np.arange(R)
    wT[col_indices, :, r_idx[:, None], :] = vals_t
    return wT.reshape(IN_F, OUT_F)


def kernel(x, values, col_indices, bias):
    global LAST_EXEC_TIME_NS
    import ml_dtypes

    _ensure_profile_hook()
    from concourse.bass_utils import run_bass_kernel_spmd

    if "nc" not in _CACHE:
        _CACHE["nc"] = _build_nc()
    nc = _CACHE["nc"]

    f16 = np.float16
    fp8 = ml_dtypes.float8_e4m3
    wT32 = _densify_wT(np.asarray(values), np.asarray(col_indices))
    xT32 = np.ascontiguousarray(np.asarray(x, dtype=np.float32).reshape(TOK, IN_F).T)
    wT = wT32.astype(f16)
    xT = xT32.astype(f16)
    bias_f = np.asarray(bias, dtype=np.float32)

    def _pack8(src32, n_t, width):
        # [n_t*128 rows, width] fp32 -> [128, n_t, width] fp8 with
        # [p, t, :] = row t*128+p (must match the device (p, t) map).
        return np.ascontiguousarray(
            src32.astype(fp8).reshape(n_t, 128, width).transpose(1, 0, 2)
        )

    in_maps = []
    for core in range(8):
        t, h = divmod(core, OUT_SHARDS)
        xs = slice(t * TOK_PER, (t + 1) * TOK_PER)
        ws = slice(h * OUT_PER, (h + 1) * OUT_PER)
        x8_src = xT32[: N_FP8 * 128, xs] * (1.0 / FP8_SCALE)
        w8_src = wT32[: N_FP8 * 128, ws] * FP8_SCALE
        in_maps.append(
            {
                "xT": np.ascontiguousarray(xT[:, xs]),
                "w": np.ascontiguousarray(wT[:, ws]),
                "x8a": _pack8(x8_src[0:256], 2, TOK_PER),
                "x8b": _pack8(x8_src[256:512], 2, TOK_PER),
                "w8La": _pack8(w8_src[:, 0 : OUT_PER // 4], N_FP8, OUT_PER // 4),
                "w8Lb": _pack8(
                    w8_src[:, OUT_PER // 4 : OUT_PER // 2], N_FP8, OUT_PER // 4
                ),
                "w8R": _pack8(w8_src[:, OUT_PER // 2 : OUT_PER], N_FP8, OUT_PER // 2),
            }
        )

    res = run_bass_kernel_spmd(
        nc,
        in_maps,
        list(range(8)),
        trace=bool(os.environ.get("BASS_TRACE")),
    )
    LAST_EXEC_TIME_NS = res.exec_time_ns

    y = np.empty((TOK, OUT_F), dtype=np.float32)
    for core in range(8):
        t, h = divmod(core, OUT_SHARDS)
        # [128, 8, TOK_PER] with col-groups g -> m = [0,2,4,6,1,3,5,7][g]
        y_dev = (
            res.results[core]["y"]
            .astype(np.float32)
            .reshape(128, M_CHUNKS, TOK_PER)
            .transpose(1, 0, 2)  # [g, p, t]
        )
        y_log = y_dev[[0, 4, 1, 5, 2, 6, 3, 7]].reshape(OUT_PER, TOK_PER)
        y[t * TOK_PER : (t + 1) * TOK_PER, h * OUT_PER : (h + 1) * OUT_PER] = y_log.T
    return (y + bias_f[None, :]).reshape(BATCH, SEQ, OUT_F)


# revision 9
# speedup vs baseline: 1.0900x; 1.0900x over previous
"""CMSBlockLinear block-ELL sparse linear forward on 8 trn2 NeuronCores.

Strategy: the block-sparse weight (R=128 x K=32 active 16x16 tiles, 25%
density) is densified on the host into W^T [2048 in, 2048 out].  The device
runs a dense matmul y^T = W^T.T @ x^T with fp32 PSUM accumulation.
Dense-ifying costs 4x the weight FLOPs on paper, but the PE streams N
columns per matmul regardless of M, so a dense 128-wide M uses the array 8x
better than the natural M=16 sparse formulation.

Sharding (8 cores): 4-way over tokens x 2-way over output features.

v2 numeric config (error budget vs the 2e-2 gate, host-sim calibrated to
5 digits against HW at the previous 2-chunk config):
- Contraction chunks 0-3 ride in fp8(e4m3) as TWO DoubleRow pair passes
  (2 k-tiles per instruction, double-pumped PE): 4 chunks of progress for
  2 chunks of PE cycles and half the DMA bytes.
- Everything else (chunks 4-15, and the output y) is fp16 instead of bf16:
  same PE rate and DMA bytes, 8x less quantization error, which buys the
  margin for the second fp8 pair.  w8 is scaled x4 and x8 by 1/4 (exact
  powers of two, product invariant) to pull w's small values out of e4m3's
  subnormal range.  Host-sim predicted rel_err 1.897e-2.

Device loop (trace-driven rework of the 44.4us v1):
- Input DMAs are need-ordered on the two HWDGE rings: sync carries
  [x8a(pair1), w8R, x8b(pair2), then x chunks 4..15], scalar carries
  [w8L, then w chunks 4..15].  w8 is split by output half so pass A's
  m=0..3 only waits on w8L+x8a; x8b/w8R land during pass A so pass B
  follows gaplessly (v1 lost 938ns waiting for its second fp8 piece).
- Warm-up dummy matmuls hold the PE (and ramp the DVFS clock) until the
  first data lands.  v1's 10-deep chain overshot the data by ~1.9us at
  the cold 1.2GHz clock; 6 slots end ~10.6us, right at the measured
  first-sem window.
- Steady state is per-chunk demand-paced fp16 DMAs (x on sync, w on
  scalar, buffers rotating 5/6-deep): front-loading everything at once
  keeps ~300GB/s in flight through the clock ramp and the DVFS governor
  then parks the PE at 2.0GHz instead of 2.4 for the whole stream.
- bias is applied on the host (zeros in this problem, exact in fp32
  either way).  Epilogue: last three chunks m-major so bank m closes
  ~0.65us before bank m+1 and the psum copies (even m on DVE, odd m on
  Scalar-ACT) + output DMAs hide under the stream tail; m7's final piece
  has its copy AND its DMA each split across both engines/rings.
- ~7.2us NEFF entry + ~3us exit (semaphore resets/barriers) are inside
  the measured window and fixed.
"""

import os

import numpy as np

BATCH, SEQ = 4, 512
IN_F = OUT_F = 2048
B = 16
R = 128  # output block rows
C = 128  # input block cols
KBLK = 32  # active tiles per row

TOK = BATCH * SEQ  # 2048 tokens
TOK_SHARDS = 4
OUT_SHARDS = 2
TOK_PER = TOK // TOK_SHARDS  # 512
OUT_PER = OUT_F // OUT_SHARDS  # 1024
K_CHUNKS = IN_F // 128  # 16
M_CHUNKS = OUT_PER // 128  # 8
N_FP8 = 4  # contraction chunks 0..3 in fp8 as two DoubleRow pairs
FP8_SCALE = 4.0  # w8 *= 4, x8 /= 4: rebalance e4m3 subnormal loss

# Warm slots bridge the first-DMA wait: the chain runs at the cold ~1.2GHz
# clock (~427ns per 512-wide matmul).  Measured body-relative: PE enters the
# body at ~body+0.65us and pass-A data is consumable at ~body+4.4us (queue
# start ~body+1.7, critical 256KB, ~1.9us completion-sem latency), so 8
# slots (~3.6us) bridge it.  PE idle mid-ramp delays the 2.4GHz grant by
# about the idle length, so the chain should end at (not before) arrival.
N_WARM = 8

LAST_EXEC_TIME_NS = None

_CACHE = {}


def _ensure_profile_hook():
    """Provide antenv.axon_hooks if the image lacks it, so trace=True works.

    Mirrors trn_agent_boot._ntff_profile_via_ctypes: drives NTFF capture via
    the libaxon_pjrt.so C ABI.  Also makes upload_artifacts fall back to the
    local dir when no artifact store is reachable.
    """
    import contextlib
    import ctypes
    import sys
    import types

    try:
        import antenv.axon_hooks  # noqa: F401

        return
    except ImportError:
        pass

    so_path = "/opt/axon/libaxon_pjrt.so"
    _hook = None
    if os.path.exists(so_path):
        try:
            lib = ctypes.CDLL(so_path)
            if hasattr(lib, "axon_start_nrt_profile"):
                lib.axon_start_nrt_profile.argtypes = [
                    ctypes.POINTER(ctypes.c_int64),
                    ctypes.c_size_t,
                ]
                lib.axon_start_nrt_profile.restype = ctypes.c_int64
                lib.axon_stop_nrt_profile.argtypes = [ctypes.c_char_p]
                lib.axon_stop_nrt_profile.restype = ctypes.c_int64

                @contextlib.contextmanager
                def _ntff_hook(output_dir, device_ids):
                    import jax

                    jax.devices()
                    if device_ids:
                        ids = (ctypes.c_int64 * len(device_ids))(*device_ids)
                        rc = lib.axon_start_nrt_profile(ids, len(device_ids))
                    else:
                        rc = lib.axon_start_nrt_profile(None, 0)
                    if rc != 0:
                        raise RuntimeError(f"axon_start_nrt_profile rc={rc}")
                    try:
                        yield
                    finally:
                        n = lib.axon_stop_nrt_profile(str(output_dir).encode())
                        print(f"profile: {n} file(s) -> {output_dir}", file=sys.stderr)

                _hook = _ntff_hook
        except OSError:
            pass

    mod = types.ModuleType("antenv.axon_hooks")
    mod.get_axon_ntff_profile_hook = lambda: _hook
    sys.modules["antenv.axon_hooks"] = mod

    import concourse.bass_utils as _bu

    _orig_upload = _bu.upload_artifacts

    def _safe_upload(tmpdir):
        try:
            return _orig_upload(tmpdir)
        except Exception:
            return tmpdir

    _bu.upload_artifacts = _safe_upload


def _build_nc():
    import concourse.mybir as mybir
    from concourse import bacc
    from concourse.tile import TileContext

    f16 = mybir.dt.float16
    fp8 = mybir.dt.float8e4

    nc = bacc.Bacc("TRN2", target_bir_lowering=False)
    xT = nc.dram_tensor("xT", [IN_F, TOK_PER], f16, kind="ExternalInput")
    w = nc.dram_tensor("w", [IN_F, OUT_PER], f16, kind="ExternalInput")
    # fp8 pieces.  Layout [p, t, :] with t = chunk index within the group;
    # lhsT/rhs agree so the DoubleRow (p, t) reduction maps correctly.
    # w8 is split by output-column half so pass A's first matmuls only wait
    # on w8L; x8 by pair so pass A only waits on x8a.
    x8a = nc.dram_tensor("x8a", [128, 2, TOK_PER], fp8, kind="ExternalInput")
    x8b = nc.dram_tensor("x8b", [128, 2, TOK_PER], fp8, kind="ExternalInput")
    Q = OUT_PER // 4  # 256 output cols = 2 m-chunks per w8 quarter piece
    w8La = nc.dram_tensor("w8La", [128, N_FP8, Q], fp8, kind="ExternalInput")
    w8Lb = nc.dram_tensor("w8Lb", [128, N_FP8, Q], fp8, kind="ExternalInput")
    w8R = nc.dram_tensor("w8R", [128, N_FP8, OUT_PER // 2], fp8, kind="ExternalInput")
    # y device layout: [partition, col-group, token] with col-groups
    # [m0,m2,m4,m6,m1,m3,m5,m7] - 1-2 KB contiguous per (partition, push).
    # Host un-permutes.
    y = nc.dram_tensor("y", [128, M_CHUNKS * TOK_PER], f16, kind="ExternalOutput")

    with TileContext(nc) as tc:
        with (
            tc.tile_pool(name="consts", bufs=1) as consts,
            tc.tile_pool(name="xp", bufs=5) as xp,
            tc.tile_pool(name="wp", bufs=6) as wp,
            tc.tile_pool(name="op", bufs=1) as op,
            tc.tile_pool(name="ps", bufs=1, space="PSUM") as ps,
        ):
            psums = [
                ps.tile([128, TOK_PER], mybir.dt.float32, tag=f"ps{m}", name=f"ps{m}")
                for m in range(M_CHUNKS)
            ]

            # Warm-up: dummy matmuls hold the PE busy (and ramp the DVFS
            # clock) until pass A's data lands.  Contents irrelevant (pass
            # A's start=True resets each bank), but Tile needs a writer to
            # allocate the tile - one cheap column memset suffices.
            warm = consts.tile([128, TOK_PER], f16)
            nc.vector.memset(warm[:, :1], 0)
            for i in range(N_WARM):
                nc.tensor.matmul(
                    psums[0][:],
                    warm[:, :128],
                    warm[:],
                    start=(i == 0),
                    stop=(i == N_WARM - 1),
                )

            # Input DMAs.  The fp16 chunks open the stream (the opener only
            # gates on xk4+wk4 = 384KB), so they go first on the rings; the
            # fp8 pieces are ALLOCATED LAST in each pool so buffer-reuse
            # pacing naturally lands their 768KB mid-stream, when the rings
            # have slack — the fp8 pair passes run in the epilogue.  Every
            # chunk in its own resident buffer, rotation 5/6-deep paces the
            # stream for DVFS.
            xks, wks = [], []
            for k in range(K_CHUNKS):
                if k < N_FP8:
                    xks.append(None)
                    wks.append(None)
                    continue
                xk = xp.tile([128, TOK_PER], f16, name=f"xk{k}", tag="xk")
                wk = wp.tile([128, OUT_PER], f16, name=f"wk{k}", tag="wk")
                nc.sync.dma_start(xk[:], xT[k * 128 : (k + 1) * 128, :])
                nc.scalar.dma_start(wk[:], w[k * 128 : (k + 1) * 128, :])
                xks.append(xk)
                wks.append(wk)
            x8at = xp.tile([128, 2, TOK_PER], fp8, name="x8at", tag="x8at")
            x8bt = xp.tile([128, 2, TOK_PER], fp8, name="x8bt", tag="x8bt")
            w8Lat = wp.tile([128, N_FP8, Q], fp8, name="w8Lat", tag="w8Lat")
            w8Lbt = wp.tile([128, N_FP8, Q], fp8, name="w8Lbt", tag="w8Lbt")
            w8Rt = wp.tile([128, N_FP8, OUT_PER // 2], fp8, name="w8Rt", tag="w8Rt")
            nc.sync.dma_start(x8at[:], x8a[:])
            nc.sync.dma_start(x8bt[:], x8b[:])
            nc.scalar.dma_start(w8Lat[:], w8La[:])
            nc.scalar.dma_start(w8Lbt[:], w8Lb[:])
            nc.scalar.dma_start(w8Rt[:], w8R[:])

            # Steady state: k-outer, m-inner (fp16 chunks 4..12).  The first
            # chunk's start=True clears each bank after the warm junk.
            for k in range(N_FP8, K_CHUNKS - 3):
                for m in range(M_CHUNKS):
                    nc.tensor.matmul(
                        psums[m][:],
                        wks[k][:, m * 128 : (m + 1) * 128],
                        xks[k][:],
                        start=(k == N_FP8),
                        stop=False,
                    )

            outA = op.tile([128, M_CHUNKS // 2, TOK_PER], f16, name="outA")
            outB = op.tile([128, M_CHUNKS // 2, TOK_PER], f16, name="outB")

            # Epilogue: last three fp16 chunks PLUS the two fp8 DoubleRow
            # pair passes, m-major, so bank m closes ~1us before bank m+1;
            # each bank's copy and each output DMA push is emitted right
            # behind its close and overlaps the stream tail.
            T = TOK_PER
            H = TOK_PER // 2
            for m in range(M_CHUNKS):
                for kk in range(K_CHUNKS - 3, K_CHUNKS):
                    nc.tensor.matmul(
                        psums[m][:],
                        wks[kk][:, m * 128 : (m + 1) * 128],
                        xks[kk][:],
                        start=False,
                        stop=False,
                    )
                w8t = (w8Lat, w8Lbt, w8Rt, w8Rt)[m // 2]
                mm = m % 2 if m < 4 else m % 4
                for pair, x8t in ((0, x8at), (1, x8bt)):
                    t0 = 2 * pair
                    nc.tensor.matmul(
                        psums[m][:],
                        w8t[:, t0 : t0 + 2, mm * 128 : (mm + 1) * 128],
                        x8t[:],
                        start=False,
                        stop=(pair == 1),
                        perf_mode=mybir.MatmulPerfMode.DoubleRow,
                    )
                j = m // 2
                if m == M_CHUNKS - 1:
                    # Split the last bank's copy across both engines to
                    # halve the post-stream copy latency.
                    nc.vector.tensor_scalar_add(outB[:, j, 0:H], psums[m][:, 0:H], 0.0)
                    nc.scalar.copy(outB[:, j, H:T], psums[m][:, H:T])
                elif m % 2 == 0:
                    nc.vector.tensor_scalar_add(outA[:, j, :], psums[m][:], 0.0)
                else:
                    nc.scalar.copy(outB[:, j, :], psums[m][:])

                if m == 2:
                    nc.sync.dma_start(y[:, 0 : 2 * T], outA[:, 0:2, :])  # m0,m2
                elif m == 3:
                    nc.scalar.dma_start(y[:, 4 * T : 6 * T], outB[:, 0:2, :])  # m1,m3
                elif m == 5:
                    nc.sync.dma_start(y[:, 6 * T : 7 * T], outB[:, 2:3, :])  # m5
                elif m == 6:
                    nc.sync.dma_start(y[:, 2 * T : 4 * T], outA[:, 2:4, :])  # m4,m6
                elif m == M_CHUNKS - 1:
                    # m7's 128 KB split across both rings, each half pushed
                    # right behind its own copy, so the final drain is two
                    # parallel 64 KB transfers instead of one serial piece.
                    nc.sync.dma_start(y[:, 7 * T : 7 * T + H], outB[:, 3, 0:H])
                    nc.scalar.dma_start(y[:, 7 * T + H : 8 * T], outB[:, 3, H:T])

    nc.finalize()
    return nc


def _densify_wT(values: np.ndarray, col_indices: np.ndarray) -> np.ndarray:
    """W^T [in=2048, out=2048] with W[r*16+i, c*16+j] = values[r,k,i,j]."""
    wT = np.zeros((C, B, R, B), dtype=np.float32)  # [c, j, r, i]
    vals_t = values.transpose(0, 1, 3, 2)  # [R, K, j, i]
    r_idx = np.arange(R)
    wT[col_indices, :, r_idx[:, None], :] = vals_t
    return wT.reshape(IN_F, OUT_F)


def kernel(x, values, col_indices, bias):
    global LAST_EXEC_TIME_NS
    import ml_dtypes

    _ensure_profile_hook()
    from concourse.bass_utils import run_bass_kernel_spmd

    if "nc" not in _CACHE:
        _CACHE["nc"] = _build_nc()
    nc = _CACHE["nc"]

    f16 = np.float16
    fp8 = ml_dtypes.float8_e4m3
    wT32 = _densify_wT(np.asarray(values), np.asarray(col_indices))
    xT32 = np.ascontiguousarray(np.asarray(x, dtype=np.float32).reshape(TOK, IN_F).T)
    wT = wT32.astype(f16)
    xT = xT32.astype(f16)
    bias_f = np.asarray(bias, dtype=np.float32)

    def _pack8(src32, n_t, width):
        # [n_t*128 rows, width] fp32 -> [128, n_t, width] fp8 with
        # [p, t, :] = row t*128+p (must match the device (p, t) map).
        return np.ascontiguousarray(
            src32.astype(fp8).reshape(n_t, 128, width).transpose(1, 0, 2)
        )

    in_maps = []
    for core in range(8):
        t, h = divmod(core, OUT_SHARDS)
        xs = slice(t * TOK_PER, (t + 1) * TOK_PER)
        ws = slice(h * OUT_PER, (h + 1) * OUT_PER)
        x8_src = xT32[: N_FP8 * 128, xs] * (1.0 / FP8_SCALE)
        w8_src = wT32[: N_FP8 * 128, ws] * FP8_SCALE
        in_maps.append(
            {
                "xT": np.ascontiguousarray(xT[:, xs]),
                "w": np.ascontiguousarray(wT[:, ws]),
                "x8a": _pack8(x8_src[0:256], 2, TOK_PER),
                "x8b": _pack8(x8_src[256:512], 2, TOK_PER),
                "w8La": _pack8(w8_src[:, 0 : OUT_PER // 4], N_FP8, OUT_PER // 4),
                "w8Lb": _pack8(
                    w8_src[:, OUT_PER // 4 : OUT_PER // 2], N_FP8, OUT_PER // 4
                ),
                "w8R": _pack8(w8_src[:, OUT_PER // 2 : OUT_PER], N_FP8, OUT_PER // 2),
            }
        )

    res = run_bass_kernel_spmd(
        nc,
        in_maps,
        list(range(8)),
        trace=bool(os.environ.get("BASS_TRACE")),
    )
    LAST_EXEC_TIME_NS = res.exec_time_ns

    y = np.empty((TOK, OUT_F), dtype=np.float32)
    for core in range(8):
        t, h = divmod(core, OUT_SHARDS)
        # [128, 8, TOK_PER] with col-groups g -> m = [0,2,4,6,1,3,5,7][g]
        y_dev = (
            res.results[core]["y"]
            .astype(np.float32)
            .reshape(128, M_CHUNKS, TOK_PER)
            .transpose(1, 0, 2)  # [g, p, t]
        )
        y_log = y_dev[[0, 4, 1, 5, 2, 6, 3, 7]].reshape(OUT_PER, TOK_PER)
        y[t * TOK_PER : (t + 1) * TOK_PER, h * OUT_PER : (h + 1) * OUT_PER] = y_log.T
    return (y + bias_f[None, :]).reshape(BATCH, SEQ, OUT_F)
